# revision 7
# baseline (speedup 1.0000x reference)
"""CrossSessionCenterAlignmentLoss on 8 Trainium2 cores.

Math: with gid = label*S + session in [0,8):
  sums_g  = sum_{i in g} f_i          -> centers c_g = sums_g / count_g
  U_g     = sum_{i in g} f_i / max(||f_i||, eps)
  sum_i cos(f_i, c_{gid_i}) = sum_g <U_g, c_g / max(||c_g||, eps)>
so ONE streaming pass over features yields everything; the final losses
are computed on the host from 8 groups x 128 dims of partial sums.

On-chip per 128-sample tile: PE matmul  out[16, 256] += oh16^T @ [hi|lo]
where oh16 = [onehot(gid) | onehot(gid)*inv_norm] (bf16, stationary) and
[hi|lo] is the bf16 hi/lo split of the f32 features (moving, N=256,
1 cycle/row).  PSUM accumulates in f32, so sums are exact to ~2^-17.
Row norms: ACT Square+accum_out per tile, ACT Sqrt + DVE reciprocal per
W-tile block (Square and Sqrt share an ACT table -> no table switches).
"""

from contextlib import ExitStack

import ml_dtypes
import numpy as np

L = 2
S = 4
NG = L * S  # 8 groups
D = 128
P = 128
EPS = 1e-8
N_CORES = 8
B = 1048576
B_LOCAL = B // N_CORES  # 131072
T = B_LOCAL // P  # 1024 tiles per core
W = 8  # tiles per norm block

_NC_CACHE = {}


def _build_nc(n_tiles):
    import concourse.bacc as bacc
    import concourse.tile as tile
    from concourse import mybir

    f32 = mybir.dt.float32
    bf16 = mybir.dt.bfloat16
    AF = mybir.ActivationFunctionType
    ALU = mybir.AluOpType

    nc = bacc.Bacc()
    fhl = nc.dram_tensor("fhl", [n_tiles, P, 2 * D], bf16, kind="ExternalInput")
    gidt = nc.dram_tensor("gidt", [P, n_tiles], mybir.dt.int8, kind="ExternalInput")
    out = nc.dram_tensor("partials", [16, 2 * D], f32, kind="ExternalOutput")

    with ExitStack() as ctx:
        tc = ctx.enter_context(tile.TileContext(nc))
        singles = ctx.enter_context(tc.tile_pool(name="singles", bufs=1))
        fpool = ctx.enter_context(tc.tile_pool(name="f", bufs=3 * W))
        sqpool = ctx.enter_context(tc.tile_pool(name="sq", bufs=2))
        statpool = ctx.enter_context(tc.tile_pool(name="stat", bufs=4))
        ohpool = ctx.enter_context(tc.tile_pool(name="oh", bufs=8))
        psump = ctx.enter_context(tc.tile_pool(name="psum", bufs=1, space="PSUM"))

        gid_i8 = singles.tile([P, n_tiles], mybir.dt.int8)
        nc.sync.dma_start(out=gid_i8[:], in_=gidt[:, :])
        # convert on DVE so every consumer's deps are DVE-internal (the
        # TT/TS instruction encodings only fit ONE embedded sync wait)
        gid_sb = singles.tile([P, n_tiles], f32)
        nc.vector.tensor_copy(out=gid_sb[:], in_=gid_i8[:])
        iota = singles.tile([P, NG], f32)
        for g in range(NG):
            nc.vector.memset(iota[:, g : g + 1], float(g))
        epsb = singles.tile([P, 1], f32)
        nc.vector.memset(epsb[:], 1e-16)

        acc = psump.tile([16, 2 * D], f32)

        n_blk = n_tiles // W
        for b in range(n_blk):
            ss = statpool.tile([P, W], f32, tag="ss")
            ftiles = []
            for wi in range(W):
                t = b * W + wi
                ft = fpool.tile([P, 2 * D], bf16, tag="f")
                nc.sync.dma_start(out=ft[:], in_=fhl[t, :, :])
                sq = sqpool.tile([P, D], bf16, tag="sq")
                nc.scalar.activation(
                    out=sq[:],
                    in_=ft[:, 0:D],
                    func=AF.Square,
                    accum_out=ss[:, wi : wi + 1],
                )
                ftiles.append(ft)
            inv = statpool.tile([P, W], f32, tag="inv")
            # inv = 1 / sqrt(ss + 1e-16)  ==  1 / max(||f||, 1e-8) up to fp32
            nc.scalar.activation(out=inv[:], in_=ss[:], func=AF.Sqrt, bias=epsb[:])
            nc.vector.reciprocal(inv[:], inv[:])
            for wi in range(W):
                t = b * W + wi
                oh = ohpool.tile([P, 2 * NG], bf16, tag="oh")
                nc.vector.tensor_tensor(
                    out=oh[:, 0:NG],
                    in0=iota[:],
                    in1=gid_sb[:, t : t + 1].to_broadcast([P, NG]),
                    op=ALU.is_equal,
                )
                nc.vector.tensor_tensor(
                    out=oh[:, NG : 2 * NG],
                    in0=oh[:, 0:NG],
                    in1=inv[:, wi : wi + 1].to_broadcast([P, NG]),
                    op=ALU.mult,
                )
                nc.tensor.matmul(
                    acc[:],
                    lhsT=oh[:],
                    rhs=ftiles[wi][:],
                    start=(t == 0),
                    stop=(t == n_tiles - 1),
                )

        osb = singles.tile([16, 2 * D], f32)
        nc.vector.tensor_copy(out=osb[:], in_=acc[:])
        nc.sync.dma_start(out=out[:, :], in_=osb[:])
    nc.compile()
    return nc


def _get_nc(n_tiles):
    if n_tiles not in _NC_CACHE:
        _NC_CACHE[n_tiles] = _build_nc(n_tiles)
    return _NC_CACHE[n_tiles]


def _host_prep(features, labels, sessions):
    bf16 = ml_dtypes.bfloat16
    f = np.ascontiguousarray(features, dtype=np.float32)
    gid = (labels.astype(np.int64) * S + sessions.astype(np.int64)).astype(np.int32)
    counts = np.bincount(gid, minlength=NG).astype(np.float64)

    f4 = f.reshape(N_CORES, T, P, D)
    hi = f4.astype(bf16)
    lo = (f4 - hi.astype(np.float32)).astype(bf16)
    fhl = np.concatenate([hi, lo], axis=-1)  # [cores, T, P, 2D]

    gidt = (
        gid.reshape(N_CORES, T, P).transpose(0, 2, 1).astype(np.int8)
    )  # [cores, P, T]
    gidt = np.ascontiguousarray(gidt)
    return fhl, gidt, counts, gid


def _host_epilogue(partials, counts):
    """partials: list of [16, 256] f32 per core."""
    sums = np.zeros((NG, D), np.float64)
    U = np.zeros((NG, D), np.float64)
    for p in partials:
        pd = p.astype(np.float64)
        sums += pd[0:NG, 0:D] + pd[0:NG, D : 2 * D]
        U += pd[NG : 2 * NG, 0:D] + pd[NG : 2 * NG, D : 2 * D]

    centers = sums / counts[:, None]
    cn = np.maximum(np.linalg.norm(centers, axis=-1), EPS)
    chat = centers / cn[:, None]
    mean_cos = float((U * chat).sum()) / B
    center_loss = 1.0 - mean_cos

    centers_ls = centers.reshape(L, S, D)
    proto = centers_ls.mean(axis=1)  # [L, D]
    nls = np.maximum(np.linalg.norm(centers_ls, axis=-1), EPS)  # [L, S]
    npr = np.maximum(np.linalg.norm(proto, axis=-1), EPS)  # [L]
    cosv = (centers_ls * proto[:, None, :]).sum(-1) / (nls * npr[:, None])
    per_class = (1.0 - cosv).sum(axis=1)  # [L]
    align_loss = 0.0
    for y in range(L):
        align_loss = (align_loss + per_class[y]) / S

    total = center_loss + align_loss
    return (
        np.float32(total),
        np.float32(center_loss),
        np.float32(align_loss),
    )


def kernel(features, labels, sessions):
    from concourse import bass_utils

    features = np.asarray(features)
    labels = np.asarray(labels)
    sessions = np.asarray(sessions)
    assert features.shape == (B, D), features.shape

    fhl, gidt, counts, _ = _host_prep(features, labels, sessions)

    nc = _get_nc(T)
    in_maps = [{"fhl": fhl[c], "gidt": gidt[c]} for c in range(N_CORES)]
    res = bass_utils.run_bass_kernel_spmd(nc, in_maps, core_ids=list(range(N_CORES)))
    partials = [r["partials"] for r in res.results]
    return _host_epilogue(partials, counts)


# revision 10
# speedup vs baseline: 134.7738x; 134.7738x over previous
"""CrossSessionCenterAlignmentLoss on 8 Trainium2 cores.

Math: with gid = label*S + session in [0,8):
  sums_g  = sum_{i in g} f_i          -> centers c_g = sums_g / count_g
  U_g     = sum_{i in g} f_i / max(||f_i||, eps)
  sum_i cos(f_i, c_{gid_i}) = sum_g <U_g, c_g / max(||c_g||, eps)>
so ONE streaming pass over features yields everything; the final losses
are computed on the host from 8 groups x 128 dims of partial sums.

On-chip per 128-sample tile: PE matmul  out[16, 256] += oh16^T @ [hi|lo]
where oh16 = [onehot(gid) | onehot(gid)*inv_norm] (bf16, stationary) and
[hi|lo] is the bf16 hi/lo split of the f32 features (moving, N=256,
1 cycle/row).  PSUM accumulates in f32, so sums are exact to ~2^-17.
Row norms: ACT Square+accum_out per tile, ACT Sqrt + DVE reciprocal per
W-tile block (Square and Sqrt share an ACT table -> no table switches).
"""

from contextlib import ExitStack

import ml_dtypes
import numpy as np

L = 2
S = 4
NG = L * S  # 8 groups
D = 128
P = 128
EPS = 1e-8
N_CORES = 8
B = 1048576
B_LOCAL = B // N_CORES  # 131072
T = B_LOCAL // P  # 1024 tiles per core
W = 8  # tiles per norm block

_NC_CACHE = {}


def _build_nc(n_tiles, repeats=1):
    import concourse.bacc as bacc
    import concourse.tile as tile
    from concourse import mybir

    f32 = mybir.dt.float32
    bf16 = mybir.dt.bfloat16
    AF = mybir.ActivationFunctionType
    ALU = mybir.AluOpType

    nc = bacc.Bacc()
    fhl = nc.dram_tensor("fhl", [n_tiles, P, 2 * D], bf16, kind="ExternalInput")
    gidt = nc.dram_tensor("gidt", [P, n_tiles], mybir.dt.int8, kind="ExternalInput")
    out = nc.dram_tensor("partials", [16, 2 * D], f32, kind="ExternalOutput")

    with ExitStack() as ctx:
        tc = ctx.enter_context(tile.TileContext(nc))
        singles = ctx.enter_context(tc.tile_pool(name="singles", bufs=1))
        fpool = ctx.enter_context(tc.tile_pool(name="f", bufs=3 * W))
        sqpool = ctx.enter_context(tc.tile_pool(name="sq", bufs=2))
        statpool = ctx.enter_context(tc.tile_pool(name="stat", bufs=4))
        ohpool = ctx.enter_context(tc.tile_pool(name="oh", bufs=8))
        psump = ctx.enter_context(tc.tile_pool(name="psum", bufs=1, space="PSUM"))

        gid_i8 = singles.tile([P, n_tiles], mybir.dt.int8)
        nc.sync.dma_start(out=gid_i8[:], in_=gidt[:, :])
        # convert on DVE so every consumer's deps are DVE-internal (the
        # TT/TS instruction encodings only fit ONE embedded sync wait)
        gid_sb = singles.tile([P, n_tiles], f32)
        nc.vector.tensor_copy(out=gid_sb[:], in_=gid_i8[:])
        iota = singles.tile([P, NG], f32)
        for g in range(NG):
            nc.vector.memset(iota[:, g : g + 1], float(g))
        epsb = singles.tile([P, 1], f32)
        nc.vector.memset(epsb[:], 1e-16)

        acc = psump.tile([16, 2 * D], f32)

        def one_pass():
            n_blk = n_tiles // W
            for b in range(n_blk):
                ss = statpool.tile([P, W], f32, tag="ss")
                ftiles = []
                for wi in range(W):
                    t = b * W + wi
                    ft = fpool.tile([P, 2 * D], bf16, tag="f")
                    nc.sync.dma_start(out=ft[:], in_=fhl[t, :, :])
                    sq = sqpool.tile([P, D], bf16, tag="sq")
                    nc.scalar.activation(
                        out=sq[:],
                        in_=ft[:, 0:D],
                        func=AF.Square,
                        accum_out=ss[:, wi : wi + 1],
                    )
                    ftiles.append(ft)
                inv = statpool.tile([P, W], f32, tag="inv")
                # inv = 1/sqrt(ss + 1e-16)  ==  1/max(||f||, 1e-8) up to fp32
                nc.scalar.activation(out=inv[:], in_=ss[:], func=AF.Sqrt, bias=epsb[:])
                nc.vector.reciprocal(inv[:], inv[:])
                for wi in range(W):
                    t = b * W + wi
                    oh = ohpool.tile([P, 2 * NG], bf16, tag="oh")
                    nc.vector.tensor_tensor(
                        out=oh[:, 0:NG],
                        in0=iota[:],
                        in1=gid_sb[:, t : t + 1].to_broadcast([P, NG]),
                        op=ALU.is_equal,
                    )
                    nc.vector.tensor_tensor(
                        out=oh[:, NG : 2 * NG],
                        in0=oh[:, 0:NG],
                        in1=inv[:, wi : wi + 1].to_broadcast([P, NG]),
                        op=ALU.mult,
                    )
                    nc.tensor.matmul(
                        acc[:],
                        lhsT=oh[:],
                        rhs=ftiles[wi][:],
                        start=(t == 0),
                        stop=(t == n_tiles - 1),
                    )

        if repeats == 1:
            one_pass()
        else:
            with tc.For_i(0, repeats, 1):
                one_pass()

        osb = singles.tile([16, 2 * D], f32)
        nc.vector.tensor_copy(out=osb[:], in_=acc[:])
        nc.sync.dma_start(out=out[:, :], in_=osb[:])
    nc.compile()
    return nc


def _get_nc(n_tiles):
    if n_tiles not in _NC_CACHE:
        _NC_CACHE[n_tiles] = _build_nc(n_tiles)
    return _NC_CACHE[n_tiles]


def _host_prep(features, labels, sessions):
    bf16 = ml_dtypes.bfloat16
    f = np.ascontiguousarray(features, dtype=np.float32)
    gid = (labels.astype(np.int64) * S + sessions.astype(np.int64)).astype(np.int32)
    counts = np.bincount(gid, minlength=NG).astype(np.float64)

    f4 = f.reshape(N_CORES, T, P, D)
    hi = f4.astype(bf16)
    lo = (f4 - hi.astype(np.float32)).astype(bf16)
    fhl = np.concatenate([hi, lo], axis=-1)  # [cores, T, P, 2D]

    gidt = (
        gid.reshape(N_CORES, T, P).transpose(0, 2, 1).astype(np.int8)
    )  # [cores, P, T]
    gidt = np.ascontiguousarray(gidt)
    return fhl, gidt, counts, gid


def _host_epilogue(partials, counts):
    """partials: list of [16, 256] f32 per core."""
    sums = np.zeros((NG, D), np.float64)
    U = np.zeros((NG, D), np.float64)
    for p in partials:
        pd = p.astype(np.float64)
        sums += pd[0:NG, 0:D] + pd[0:NG, D : 2 * D]
        U += pd[NG : 2 * NG, 0:D] + pd[NG : 2 * NG, D : 2 * D]

    centers = sums / counts[:, None]
    cn = np.maximum(np.linalg.norm(centers, axis=-1), EPS)
    chat = centers / cn[:, None]
    mean_cos = float((U * chat).sum()) / B
    center_loss = 1.0 - mean_cos

    centers_ls = centers.reshape(L, S, D)
    proto = centers_ls.mean(axis=1)  # [L, D]
    nls = np.maximum(np.linalg.norm(centers_ls, axis=-1), EPS)  # [L, S]
    npr = np.maximum(np.linalg.norm(proto, axis=-1), EPS)  # [L]
    cosv = (centers_ls * proto[:, None, :]).sum(-1) / (nls * npr[:, None])
    per_class = (1.0 - cosv).sum(axis=1)  # [L]
    align_loss = 0.0
    for y in range(L):
        align_loss = (align_loss + per_class[y]) / S

    total = center_loss + align_loss
    return (
        np.float32(total),
        np.float32(center_loss),
        np.float32(align_loss),
    )


def kernel(features, labels, sessions):
    from concourse import bass_utils

    features = np.asarray(features)
    labels = np.asarray(labels)
    sessions = np.asarray(sessions)
    assert features.shape == (B, D), features.shape

    fhl, gidt, counts, _ = _host_prep(features, labels, sessions)

    nc = _get_nc(T)
    in_maps = [{"fhl": fhl[c], "gidt": gidt[c]} for c in range(N_CORES)]
    res = bass_utils.run_bass_kernel_spmd(nc, in_maps, core_ids=list(range(N_CORES)))
    partials = [r["partials"] for r in res.results]
    return _host_epilogue(partials, counts)
